# revision 5
# baseline (speedup 1.0000x reference)
"""Gaussian falloff vortex-velocity kernel for Trainium2 (Bass/Tile), fp16 I/O.

Math per batch element b (single vortex y,x,tau,sig per batch):
    d1 = py - y;  d2 = px - x;  q = d1^2 + d2^2
    s  = tau * exp(-q/sig^2) / sqrt(q)
    out[..., 0] = s * d2;  out[..., 1] = -s * d1

The l2 gate (2e-2) leaves room for fp16 transport: points are cast to
fp16 on the host (load traffic halves), all on-chip tensors are fp16
(enables DVE 2x perf modes), and the output is stored fp16 scaled by
g2/256 per batch (host rescales; keeps everything clear of fp16
overflow).  Emulated end-to-end error: l2 ~ 8.8e-3.

Per-core layout (8 batches, each 512x512 points = [128, 2048] per coord):
    pts DRAM [8*128, 4096] fp16, row b*128+p = [PY(2048) | PX(2048)]
    out DRAM [8*128, 4096] fp16, same split = [Pe | Po], out = (Pe, -Po)*256/g2

On-chip per batch, with g2 = sqrt(2)/sig (folds the 2/sig^2 of the
exponent into the coordinate scale so u = |d*g2|^2 = 2q/sig^2 and the
Exp argument is a plain add):
    d2' = (px - x)*g2              DVE tensor_scalar (2x)
    d1' = (py - y)*g2              DVE or GpSimd tensor_scalar (balance)
    Qo  = d2'*d2'                  DVE tensor_tensor (2x)
    Qe  = Square(py*g2 - y*g2)     ACT fused affine  (or DVE TT d1'*d1'
                                   for batches in DVE_SQ - engine balance)
    u   = Qe + Qo                  GpSimd tensor_tensor (offload)
    L   = Ln(u + eps)              ACT (eps floors Ln away from -inf)
    z   = u + L                    DVE tensor_tensor (2x)
    s   = Exp(-0.5*z + ln(tau*g2/256))   ACT  ( = tau*exp(-q/sig^2)/sqrt(q) * g2/256 )
    Pe  = d2'*s ; Po = d1'*s       DVE tensor_tensor (2x)
Host: out_even = Pe*256/g2, out_odd = -Po*256/g2.

No scalar_tensor_tensor anywhere: STT has no fast DVE mode (measured
2719ns vs 1005ns for TT at [128,2048] fp16).
"""

import numpy as np

import concourse.bass as bass
import concourse.bacc as bacc
import concourse.mybir as mybir
from concourse.tile import TileContext
from concourse.bass_utils import run_bass_kernel_spmd
from concourse.hw_specs import get_activation_tables

N_CORES = 8
B_PER_CORE = 8          # 64 batches / 8 cores
P = 128                 # SBUF partitions
COLS = 2048             # points per partition for one batch (512*512/128)
NCONST = 6              # g2, -y*g2, eps, ln(tau*g2/OS), x, y
OS = 256.0              # output scale: stored = true*g2/OS
EPS = 1e-6              # Ln(u+eps) floor: keeps s finite at u->0
DVE_SQ = (6, 7)         # batches whose Square runs on DVE (engine balance)
GPS_D1 = (3, 4, 5)      # batches whose d1' tensor_scalar runs on GpSimd
GPS_ADD = True          # u = Qe+Qo on GpSimd (False: on DVE)

_PROGRAM = None


def _pin_act_table_set(arch: str):
    """Make all our activation functions resolve to the single
    `natural_log_exp_and_others` table set (one ~2.7us table load)."""
    AF = mybir.ActivationFunctionType
    try:
        tables = get_activation_tables(arch)
        keep = "natural_log_exp_and_others"
        needed = {AF.Identity, AF.Square, AF.Ln, AF.Exp, AF.Copy}
        if keep not in tables or not needed <= tables[keep]:
            return  # unexpected table layout: skip pinning (correct, slower)
        for name, fns in tables.items():
            if name != keep:
                fns -= needed
    except Exception:
        pass


def _build_program():
    f16 = mybir.dt.float16
    f32 = mybir.dt.float32
    AF = mybir.ActivationFunctionType
    OP = mybir.AluOpType

    nc = bacc.Bacc(
        "TRN2",
        target_bir_lowering=False,
        debug=False,
        num_devices=N_CORES,
    )
    _pin_act_table_set(nc.m.arch)
    pts = nc.declare_dram_parameter("points", [B_PER_CORE * P, 2 * COLS], f16, isOutput=False)
    cst = nc.declare_dram_parameter("consts", [P, NCONST * B_PER_CORE], f32, isOutput=False)
    out = nc.declare_dram_parameter("out", [B_PER_CORE * P, 2 * COLS], f16, isOutput=True)

    with TileContext(nc) as tc:
        with (
            tc.tile_pool(name="cpool", bufs=1) as cpool,
            tc.tile_pool(name="tp", bufs=5) as tp,        # T tiles (in), 1MB
            tc.tile_pool(name="d1p", bufs=4) as d1p,      # d1' tiles, 0.5MB
            tc.tile_pool(name="d2p", bufs=4) as d2p,      # d2' tiles, 0.5MB
            tc.tile_pool(name="qe", bufs=2) as qe_pool,   # Qe tiles
            tc.tile_pool(name="qq", bufs=3) as qq_pool,   # Qo -> u -> z tiles
            tc.tile_pool(name="ep", bufs=3) as ep_pool,   # L -> s tiles
            tc.tile_pool(name="op", bufs=3) as op_pool,   # O tiles (out), 1MB
        ):
            # Consts first on the sync ring (tiny, lands ahead of big loads).
            c = cpool.tile([P, NCONST * B_PER_CORE], f32)
            nc.sync.dma_start(c[:], cst[:])

            # Warm-up activation with no dependencies: pulls the ACT table
            # load off the critical path.
            w = cpool.tile([P, 1], f32)
            nc.vector.memset(w[:], 1.0)
            nc.scalar.activation(w[:], w[:], AF.Exp)

            def cap(b, j):
                return c[:, NCONST * b + j : NCONST * b + j + 1]

            # Work items: first/last batch split in halves to shorten
            # pipeline fill/drain.
            items = []
            for b in range(B_PER_CORE):
                if b in (0, B_PER_CORE - 1):
                    items.append((b, 0, COLS // 2))
                    items.append((b, COLS // 2, COLS // 2))
                else:
                    items.append((b, 0, COLS))
            NI = len(items)

            pts_v = pts[:, :].rearrange("p (h c) -> p h c", h=2)
            out_v = out[:, :].rearrange("p (h c) -> p h c", h=2)

            Ts, D1s, D2s, Qes, Qs, Ls = {}, {}, {}, {}, {}, {}

            def load(i):
                b, c0, wdt = items[i]
                rows = slice(b * P, (b + 1) * P)
                T = tp.tile([P, 2, wdt], f16, tag="T")
                nc.sync.dma_start(T[:], pts_v[rows, :, c0 : c0 + wdt])
                Ts[i] = T

            def stage_a(i):
                # d2'=(px-x)g2 ; d1'=(py-y)g2 ; Qo=d2'^2 ; Qe=Square(affine py)
                b, c0, wdt = items[i]
                T = Ts[i]
                PY, PX = T[:, 0], T[:, 1]
                d2 = d2p.tile([P, wdt], f16, tag="d2")
                nc.vector.tensor_scalar(d2[:], PX, cap(b, 4), cap(b, 0), OP.subtract, OP.mult)
                d1 = d1p.tile([P, wdt], f16, tag="d1")
                d1_eng = nc.gpsimd if b in GPS_D1 else nc.vector
                d1_eng.tensor_scalar(d1[:], PY, cap(b, 5), cap(b, 0), OP.subtract, OP.mult)
                Qo = qq_pool.tile([P, wdt], f16, tag="q")
                nc.vector.tensor_tensor(Qo[:], d2[:], d2[:], OP.mult)
                Qe = qe_pool.tile([P, wdt], f16, tag="qe")
                if b in DVE_SQ:
                    nc.vector.tensor_tensor(Qe[:], d1[:], d1[:], OP.mult)
                else:
                    nc.scalar.activation(Qe[:], PY, AF.Square, bias=cap(b, 1), scale=cap(b, 0))
                D1s[i], D2s[i], Qs[i], Qes[i] = d1, d2, Qo, Qe

            def stage_b(i):
                # u = Qe + Qo ; L = Ln(u+eps) ; z = u + L  (z over u)
                b = items[i][0]
                q, Qe = Qs[i], Qes[i]
                if GPS_ADD:
                    nc.gpsimd.tensor_tensor(q[:], q[:], Qe[:], OP.add)
                else:
                    nc.vector.tensor_tensor(q[:], q[:], Qe[:], OP.add)
                L = ep_pool.tile([P, q.shape[-1]], f16, tag="L")
                nc.scalar.activation(L[:], q[:], AF.Ln, bias=cap(b, 2))
                nc.vector.tensor_tensor(q[:], q[:], L[:], OP.add)
                Ls[i] = L
                del Qes[i]

            def stage_c(i):
                # s = Exp(-z/2 + lnC) (over L) ; Pe = d2'*s ; Po = d1'*s ; store
                b, c0, wdt = items[i]
                z, s = Qs[i], Ls[i]
                d1, d2 = D1s[i], D2s[i]
                nc.scalar.activation(s[:], z[:], AF.Exp, bias=cap(b, 3), scale=-0.5)
                O = op_pool.tile([P, 2, wdt], f16, tag="O")
                nc.vector.tensor_tensor(O[:, 0], d2[:], s[:], OP.mult)
                nc.vector.tensor_tensor(O[:, 1], d1[:], s[:], OP.mult)
                rows = slice(b * P, (b + 1) * P)
                nc.sync.dma_start(out_v[rows, :, c0 : c0 + wdt], O[:])
                del Ts[i], Qs[i], Ls[i], D1s[i], D2s[i]

            # Software pipeline: loads lead compute by one step.
            load(0)
            for t in range(NI + 2):
                if t + 1 < NI:
                    load(t + 1)
                if 1 <= t <= NI:
                    stage_b(t - 1)
                if t >= 2:
                    stage_c(t - 2)
                if t < NI:
                    stage_a(t)

    nc.compile()
    return nc


def _get_program():
    global _PROGRAM
    if _PROGRAM is None:
        _PROGRAM = _build_program()
    return _PROGRAM


def _make_in_maps(vortex_feature, points):
    B = points.shape[0]
    vf = np.asarray(vortex_feature, dtype=np.float64).reshape(B, 6)
    y, x, tau, sig = vf[:, 0], vf[:, 1], vf[:, 2], vf[:, 3]
    sig_c = np.maximum(sig, 1e-35)
    g2 = np.sqrt(2.0) / sig_c  # coordinate scale: u = |d*g2|^2 = 2q/sig^2
    with np.errstate(divide="ignore"):
        lnC = np.log(tau) + np.log(g2) - np.log(OS)  # ln(tau*g2/OS)
    consts = np.stack(
        [g2, -y * g2, np.full_like(y, EPS), lnC, x, y], axis=1
    ).astype(np.float32)

    pf16 = np.asarray(points, dtype=np.float16)  # host-side cast (free for HW)
    in_maps = []
    for i in range(N_CORES):
        sl = slice(i * B_PER_CORE, (i + 1) * B_PER_CORE)
        py = pf16[sl, :, :, 0].reshape(B_PER_CORE, P, COLS)
        px = pf16[sl, :, :, 1].reshape(B_PER_CORE, P, COLS)
        pshard = np.ascontiguousarray(
            np.stack([py, px], axis=2).reshape(B_PER_CORE * P, 2 * COLS)
        )
        cshard = np.ascontiguousarray(
            np.broadcast_to(
                consts[sl].reshape(1, NCONST * B_PER_CORE), (P, NCONST * B_PER_CORE)
            )
        )
        in_maps.append({"points": pshard, "consts": cshard})
    return in_maps, g2


def run(vortex_feature, points, trace=False, tmpdir=None):
    nc = _get_program()
    in_maps, g2 = _make_in_maps(vortex_feature, points)
    last_err = None
    for _ in range(3):
        try:
            res = run_bass_kernel_spmd(nc, in_maps, list(range(N_CORES)), trace=trace, tmpdir=tmpdir)
            break
        except Exception as err:  # noqa: BLE001
            last_err = err
    else:
        raise last_err
    B, H, W, _ = points.shape
    un = (OS / g2).astype(np.float32)  # per-batch unscale
    out = np.empty((B, H, W, 2), dtype=np.float32)
    for i in range(N_CORES):
        sl = slice(i * B_PER_CORE, (i + 1) * B_PER_CORE)
        o = res.results[i]["out"].reshape(B_PER_CORE, P, 2, COLS).astype(np.float32)
        o *= un[sl][:, None, None, None]
        out[sl, :, :, 0] = o[:, :, 0, :].reshape(B_PER_CORE, H, W)
        out[sl, :, :, 1] = -o[:, :, 1, :].reshape(B_PER_CORE, H, W)
    return out, res


def kernel(vortex_feature: np.ndarray, points: np.ndarray) -> np.ndarray:
    out, _ = run(vortex_feature, points, trace=False)
    return out


# revision 8
# speedup vs baseline: 2.3944x; 2.3944x over previous
"""Gaussian falloff vortex-velocity kernel for Trainium2 (Bass/Tile), fp16 I/O.

Math per batch element b (single vortex y,x,tau,sig per batch):
    d1 = py - y;  d2 = px - x;  q = d1^2 + d2^2
    s  = tau * exp(-q/sig^2) / sqrt(q)
    out[..., 0] = s * d2;  out[..., 1] = -s * d1

The l2 gate (2e-2) leaves room for fp16 transport: points are cast to
fp16 on the host (load traffic halves), all on-chip tensors are fp16
(enables DVE 2x perf modes), and the output is stored fp16 scaled by
g2/256 per batch (host rescales; keeps everything clear of fp16
overflow).  Emulated end-to-end error: l2 ~ 8.8e-3.

Per-core layout (8 batches, each 512x512 points = [128, 2048] per coord):
    pts DRAM [8*128, 4096] fp16, row b*128+p = [PY(2048) | PX(2048)]
    out DRAM [8*128, 4096] fp16, same split = [Pe | Po], out = (Pe, -Po)*256/g2

On-chip per batch, with g2 = sqrt(2)/sig (folds the 2/sig^2 of the
exponent into the coordinate scale so u = |d*g2|^2 = 2q/sig^2 and the
Exp argument is a plain add):
    d2' = (px - x)*g2              DVE tensor_scalar (2x)
    d1' = (py - y)*g2              DVE or GpSimd tensor_scalar (balance)
    Qo  = d2'*d2'                  DVE tensor_tensor (2x)
    Qe  = Square(py*g2 - y*g2)     ACT fused affine  (or DVE TT d1'*d1'
                                   for batches in DVE_SQ - engine balance)
    u   = Qe + Qo                  GpSimd tensor_tensor (offload)
    L   = Ln(u + eps)              ACT (eps floors Ln away from -inf)
    z   = u + L                    DVE tensor_tensor (2x)
    s   = Exp(-0.5*z + ln(tau*g2/256))   ACT  ( = tau*exp(-q/sig^2)/sqrt(q) * g2/256 )
    Pe  = d2'*s ; Po = d1'*s       DVE tensor_tensor (2x)
Host: out_even = Pe*256/g2, out_odd = -Po*256/g2.

No scalar_tensor_tensor anywhere: STT has no fast DVE mode (measured
2719ns vs 1005ns for TT at [128,2048] fp16).
"""

import numpy as np

import concourse.bass as bass
import concourse.bacc as bacc
import concourse.mybir as mybir
from concourse.tile import TileContext
from concourse.bass_utils import run_bass_kernel_spmd
from concourse.hw_specs import get_activation_tables

N_CORES = 8
B_PER_CORE = 8          # 64 batches / 8 cores
P = 128                 # SBUF partitions
COLS = 2048             # points per partition for one batch (512*512/128)
NCONST = 6              # g2, -y*g2, eps, ln(tau*g2/OS), x, y
OS = 256.0              # output scale: stored = true*g2/OS
EPS = 1e-6              # Ln(u+eps) floor: keeps s finite at u->0
DVE_SQ = (5, 6, 7)      # batches whose Square runs on DVE (engine balance)
GPS_ADD = False         # u = Qe+Qo on GpSimd (False: on DVE); GpSimd
                        # elementwise contends with DVE on the shared SBUF port
MM_CHUNK = 512          # PSUM bank width (f32) for the TensorE z-add

_PROGRAM = None
_IDENT = None


def _pin_act_table_set(arch: str):
    """Make all our activation functions resolve to the single
    `natural_log_exp_and_others` table set (one ~2.7us table load)."""
    AF = mybir.ActivationFunctionType
    try:
        tables = get_activation_tables(arch)
        keep = "natural_log_exp_and_others"
        needed = {AF.Identity, AF.Square, AF.Ln, AF.Exp, AF.Copy}
        if keep not in tables or not needed <= tables[keep]:
            return  # unexpected table layout: skip pinning (correct, slower)
        for name, fns in tables.items():
            if name != keep:
                fns -= needed
    except Exception:
        pass


def _build_program():
    f16 = mybir.dt.float16
    bf16 = mybir.dt.bfloat16
    f32 = mybir.dt.float32
    AF = mybir.ActivationFunctionType
    OP = mybir.AluOpType

    nc = bacc.Bacc(
        "TRN2",
        target_bir_lowering=False,
        debug=False,
        num_devices=N_CORES,
    )
    _pin_act_table_set(nc.m.arch)
    pts = nc.declare_dram_parameter("points", [B_PER_CORE * P, 2 * COLS], f16, isOutput=False)
    cst = nc.declare_dram_parameter("consts", [P, NCONST * B_PER_CORE], f32, isOutput=False)
    idn = nc.declare_dram_parameter("ident", [P, P], f16, isOutput=False)
    out = nc.declare_dram_parameter("out", [B_PER_CORE * P, 2 * COLS], f16, isOutput=True)

    with TileContext(nc) as tc:
        with (
            tc.tile_pool(name="cpool", bufs=1) as cpool,
            tc.tile_pool(name="tp", bufs=5) as tp,        # T tiles (in), 1MB
            tc.tile_pool(name="d1p", bufs=4) as d1p,      # d1' tiles, 0.5MB
            tc.tile_pool(name="d2p", bufs=4) as d2p,      # d2' tiles, 0.5MB
            tc.tile_pool(name="qe", bufs=2) as qe_pool,   # Qe tiles
            tc.tile_pool(name="qq", bufs=3) as qq_pool,   # Qo -> u -> z tiles
            tc.tile_pool(name="ep", bufs=3) as ep_pool,   # L -> s tiles
            tc.tile_pool(name="op", bufs=3) as op_pool,   # O tiles (out), 1MB
            tc.tile_pool(name="zp", bufs=2, space="PSUM") as zp_pool,  # z (f32)
        ):
            # Consts first on the sync ring (tiny, lands ahead of big loads).
            c = cpool.tile([P, NCONST * B_PER_CORE], f32)
            nc.sync.dma_start(c[:], cst[:])
            ident = cpool.tile([P, P], f16)
            nc.sync.dma_start(ident[:], idn[:])

            # Warm-up activation with no dependencies: pulls the ACT table
            # load off the critical path.
            w = cpool.tile([P, 1], f32)
            nc.vector.memset(w[:], 1.0)
            nc.scalar.activation(w[:], w[:], AF.Exp)

            def cap(b, j):
                return c[:, NCONST * b + j : NCONST * b + j + 1]

            # Work items: first/last batch split in halves to shorten
            # pipeline fill/drain.
            items = []
            for b in range(B_PER_CORE):
                if b in (0, B_PER_CORE - 1):
                    items.append((b, 0, COLS // 2))
                    items.append((b, COLS // 2, COLS // 2))
                else:
                    items.append((b, 0, COLS))
            NI = len(items)

            pts_v = pts[:, :].rearrange("p (h c) -> p h c", h=2)
            out_v = out[:, :].rearrange("p (h c) -> p h c", h=2)

            Ts, D1s, D2s, Qes, Qs, Ls = {}, {}, {}, {}, {}, {}

            def load(i):
                b, c0, wdt = items[i]
                rows = slice(b * P, (b + 1) * P)
                T = tp.tile([P, 2, wdt], f16, tag="T")
                nc.sync.dma_start(T[:], pts_v[rows, :, c0 : c0 + wdt])
                Ts[i] = T

            def stage_a(i):
                # d2'=(px-x)g2 ; d1'=(py-y)g2 ; Qo=d2'^2 ; Qe=Square(affine py)
                b, c0, wdt = items[i]
                T = Ts[i]
                PY, PX = T[:, 0], T[:, 1]
                d2 = d2p.tile([P, wdt], f16, tag="d2")
                nc.vector.tensor_scalar(d2[:], PX, cap(b, 4), cap(b, 0), OP.subtract, OP.mult)
                d1 = d1p.tile([P, wdt], f16, tag="d1")
                nc.vector.tensor_scalar(d1[:], PY, cap(b, 5), cap(b, 0), OP.subtract, OP.mult)
                Qo = qq_pool.tile([P, wdt], bf16, tag="q")
                nc.vector.tensor_tensor(Qo[:], d2[:], d2[:], OP.mult)
                Qe = qe_pool.tile([P, wdt], bf16, tag="qe")
                if b in DVE_SQ:
                    nc.vector.tensor_tensor(Qe[:], d1[:], d1[:], OP.mult)
                else:
                    nc.scalar.activation(Qe[:], PY, AF.Square, bias=cap(b, 1), scale=cap(b, 0))
                D1s[i], D2s[i], Qs[i], Qes[i] = d1, d2, Qo, Qe

            Zs = {}

            def stage_b(i):
                # u = Qe + Qo (GpSimd) ; L = Ln(u+eps) (ACT) ;
                # z = u + L via TensorE identity matmuls into PSUM (f32).
                b, c0, wdt = items[i]
                q, Qe = Qs[i], Qes[i]
                if GPS_ADD:
                    nc.gpsimd.tensor_tensor(q[:], q[:], Qe[:], OP.add)
                else:
                    nc.vector.tensor_tensor(q[:], q[:], Qe[:], OP.add)
                L = ep_pool.tile([P, wdt], f16, tag="L")
                nc.scalar.activation(L[:], q[:], AF.Ln, bias=cap(b, 2))
                z = zp_pool.tile([P, wdt], f32, tag="z")
                for c0m in range(0, wdt, MM_CHUNK):
                    sl = slice(c0m, c0m + MM_CHUNK)
                    nc.tensor.matmul(z[:, sl], ident[:], q[:, sl], start=True, stop=False)
                    nc.tensor.matmul(z[:, sl], ident[:], L[:, sl], start=False, stop=True)
                Zs[i] = z
                Ls[i] = L
                del Qes[i]

            def stage_c(i):
                # s = Exp(-z/2 + lnC) (over L) ; Pe = d2'*s ; Po = d1'*s ; store
                b, c0, wdt = items[i]
                z, s = Zs[i], Ls[i]
                d1, d2 = D1s[i], D2s[i]
                nc.scalar.activation(s[:], z[:], AF.Exp, bias=cap(b, 3), scale=-0.5)
                O = op_pool.tile([P, 2, wdt], f16, tag="O")
                nc.vector.tensor_tensor(O[:, 0], d2[:], s[:], OP.mult)
                nc.vector.tensor_tensor(O[:, 1], d1[:], s[:], OP.mult)
                rows = slice(b * P, (b + 1) * P)
                nc.sync.dma_start(out_v[rows, :, c0 : c0 + wdt], O[:])
                del Ts[i], Qs[i], Ls[i], Zs[i], D1s[i], D2s[i]

            # Software pipeline: loads lead compute by one step.
            load(0)
            for t in range(NI + 2):
                if t + 1 < NI:
                    load(t + 1)
                if 1 <= t <= NI:
                    stage_b(t - 1)
                if t >= 2:
                    stage_c(t - 2)
                if t < NI:
                    stage_a(t)

    nc.compile()
    return nc


def _get_program():
    global _PROGRAM
    if _PROGRAM is None:
        _PROGRAM = _build_program()
    return _PROGRAM


def _make_in_maps(vortex_feature, points):
    B = points.shape[0]
    vf = np.asarray(vortex_feature, dtype=np.float64).reshape(B, 6)
    y, x, tau, sig = vf[:, 0], vf[:, 1], vf[:, 2], vf[:, 3]
    sig_c = np.maximum(sig, 1e-35)
    g2 = np.sqrt(2.0) / sig_c  # coordinate scale: u = |d*g2|^2 = 2q/sig^2
    with np.errstate(divide="ignore"):
        lnC = np.log(tau) + np.log(g2) - np.log(OS)  # ln(tau*g2/OS)
    consts = np.stack(
        [g2, -y * g2, np.full_like(y, EPS), lnC, x, y], axis=1
    ).astype(np.float32)

    pf16 = np.asarray(points, dtype=np.float16)  # host-side cast (free for HW)
    in_maps = []
    global _IDENT
    if _IDENT is None:
        _IDENT = np.ascontiguousarray(np.eye(P, dtype=np.float16))
    for i in range(N_CORES):
        sl = slice(i * B_PER_CORE, (i + 1) * B_PER_CORE)
        py = pf16[sl, :, :, 0].reshape(B_PER_CORE, P, COLS)
        px = pf16[sl, :, :, 1].reshape(B_PER_CORE, P, COLS)
        pshard = np.ascontiguousarray(
            np.stack([py, px], axis=2).reshape(B_PER_CORE * P, 2 * COLS)
        )
        cshard = np.ascontiguousarray(
            np.broadcast_to(
                consts[sl].reshape(1, NCONST * B_PER_CORE), (P, NCONST * B_PER_CORE)
            )
        )
        in_maps.append({"points": pshard, "consts": cshard, "ident": _IDENT})
    return in_maps, g2


def run(vortex_feature, points, trace=False, tmpdir=None):
    nc = _get_program()
    in_maps, g2 = _make_in_maps(vortex_feature, points)
    last_err = None
    for _ in range(3):
        try:
            res = run_bass_kernel_spmd(nc, in_maps, list(range(N_CORES)), trace=trace, tmpdir=tmpdir)
            break
        except Exception as err:  # noqa: BLE001
            last_err = err
    else:
        raise last_err
    B, H, W, _ = points.shape
    un = (OS / g2).astype(np.float32)  # per-batch unscale
    out = np.empty((B, H, W, 2), dtype=np.float32)
    for i in range(N_CORES):
        sl = slice(i * B_PER_CORE, (i + 1) * B_PER_CORE)
        o = res.results[i]["out"].reshape(B_PER_CORE, P, 2, COLS).astype(np.float32)
        o *= un[sl][:, None, None, None]
        out[sl, :, :, 0] = o[:, :, 0, :].reshape(B_PER_CORE, H, W)
        out[sl, :, :, 1] = -o[:, :, 1, :].reshape(B_PER_CORE, H, W)
    return out, res


def kernel(vortex_feature: np.ndarray, points: np.ndarray) -> np.ndarray:
    out, _ = run(vortex_feature, points, trace=False)
    return out
